# revision 12
# baseline (speedup 1.0000x reference)
"""Trainium2 Bass kernel for nn_GaussianSelfAttention (B=64, S=197, D=768).

Math: the reference's softmax is over a singleton axis, so attn == 1.0 exactly
and out = concat([ones(B,1,D), sample_v], axis=1) where
sample_v = (G @ x) @ Wv + wsum*bv,  G = per-image (196,197) bilinear one-hot
matrix built from Gaussian-sampled keys. q/k projections are dead code.

Device strategy (8 cores, data-parallel over batch, 8 images/core):
  - host builds the per-image gather matrices GT (197x196) from the tiny
    O(B*P) index math and ships them as fp16 (~0.6MB/core)
  - everything on the PE runs in fp16 (1 cycle/row, no f32r N>=256 padding)
  - gather: sxT[d,q] = x^T-gather via matmul(lhsT=x[s,d], rhs=GT[s,p]),
    12 matmuls/image at N=196
  - projection: out^T[dout,q] = Wv^T @ sxT via matmul(lhsT=wv chunk,
    rhs=sxT, N=1568 split into 4x392 image-pair chunks) -> no ragged
    13th m-chunk, no transposes, no on-device index math at all
  - PE does only matmuls (~75k cycles); DVE/Act do PSUM->SBUF fp16 copies
  - per-image input DMAs split across the SP and Act HWDGE queues so the
    first gather starts ~2.5us in; Wv on the DVE (SWDGE) queue
"""

import numpy as np

import concourse.bass as bass
import concourse.mybir as mybir
import concourse.tile as tile
from concourse import bacc, bass_utils

B, S, D, P = 64, 197, 768, 196
N_CORES = 8
BPC = B // N_CORES            # images per core
Q = BPC * P                   # 1568 sampled rows per core
GRID = 14.0
W2 = 2 * P                    # 392: per-image GT pack width / q-chunk width
SPLIT = 98                    # s split point for the clean two-chunk gather

F16 = mybir.dt.float16
F32 = mybir.dt.float32

_NC = {}
_RUNNER = {}
_CFG = None  # per-slot (Najmin, Najmax) tuple from the last _pack_inputs
_ORDER = None  # image order (rank -> original batch index)
_PERM = None   # per-image column permutation


def _emit(nc, iters=1, cfg=None):
    x_d = nc.dram_tensor("x0", (128, 2 * BPC * D), F16, kind="ExternalInput")
    wv_d = nc.dram_tensor("wv0", (128, 6 * D), F16, kind="ExternalInput")
    gt_d = nc.dram_tensor("gt0", (128, BPC * W2), F16, kind="ExternalInput")
    o_d = nc.dram_tensor("o0", (128, 6 * Q), F16, kind="ExternalOutput")

    with tile.TileContext(nc) as tc:
        with (
            tc.tile_pool(name="xb", bufs=2) as xpool,
            tc.tile_pool(name="wvp", bufs=2) as wpool,
            tc.tile_pool(name="gtp", bufs=2) as gtpool,
            tc.tile_pool(name="sxp", bufs=2) as spool,
            tc.tile_pool(name="ost", bufs=2) as opool,
            tc.tile_pool(name="psA", bufs=4, space="PSUM") as psA,
            tc.tile_pool(name="psB", bufs=3, space="PSUM") as psB,
        ):
            def body():
                xall = xpool.tile([128, 2 * BPC * D], F16, name="xall",
                                  tag="xall")
                wvt = wpool.tile([128, 6 * D], F16, name="wvt", tag="wvt")
                gtall = gtpool.tile([128, BPC * W2], F16, name="gtall",
                                    tag="gtall")
                # per-image input DMAs, alternating SP / Act queues
                for b in range(BPC):
                    eng = nc.sync if b % 2 == 0 else nc.scalar
                    eng.dma_start(out=gtall[:, b * W2:(b + 1) * W2],
                                  in_=gt_d[:, b * W2:(b + 1) * W2])
                    eng.dma_start(out=xall[:, 2 * b * D:(2 * b + 2) * D],
                                  in_=x_d[:, 2 * b * D:(2 * b + 2) * D])
                nc.gpsimd.dma_start(out=wvt[:], in_=wv_d[:])

                ka, kc_ = (SPLIT, S - SPLIT) if cfg else (128, 69)

                def xsl(b, c, mj):   # lhsT slice of x image b, s-chunk c
                    t = 2 * b + c
                    pn = ka if c == 0 else kc_
                    return xall[0:pn, t * D + mj * 128: t * D + (mj + 1) * 128]

                sxT = [spool.tile([128, Q], F16, name=f"sxT{kc}",
                                  tag=f"sxT{kc}") for kc in range(6)]

                def gather(b):
                    g0 = gtall[0:ka, b * W2: b * W2 + P]
                    g1 = gtall[0:kc_, b * W2 + P: (b + 1) * W2]
                    for mj in range(6):
                        pa = psA.tile([128, P], F32, name="pa", tag="pa")
                        la, lc = xsl(b, 0, mj), xsl(b, 1, mj)
                        if cfg is None:
                            nc.tensor.matmul(pa[:], lhsT=la, rhs=g0,
                                             start=True, stop=False)
                            nc.tensor.matmul(pa[:], lhsT=lc, rhs=g1,
                                             start=False, stop=True)
                        else:
                            # columns [0,lo) pure chunk-A, [hi,P) pure chunk-C,
                            # [lo,hi) accumulated from both (zeros where absent)
                            lo, hi = cfg[b]
                            if lo > 0:
                                nc.tensor.matmul(pa[:, 0:lo],
                                                 lhsT=la, rhs=g0[:, 0:lo],
                                                 start=True, stop=True)
                            if hi > lo:
                                nc.tensor.matmul(pa[:, lo:hi],
                                                 lhsT=la, rhs=g0[:, lo:hi],
                                                 start=True, stop=False)
                                nc.tensor.matmul(pa[:, lo:hi],
                                                 lhsT=lc, rhs=g1[:, lo:hi],
                                                 start=False, stop=True)
                            if hi < P:
                                nc.tensor.matmul(pa[:, hi:P],
                                                 lhsT=lc, rhs=g1[:, hi:P],
                                                 start=True, stop=True)
                        eng = nc.vector.tensor_copy if mj % 2 else nc.scalar.copy
                        eng(out=sxT[mj][:, b * P:(b + 1) * P], in_=pa[:])

                ot = [opool.tile([128, Q], F16, name=f"ot{m}", tag=f"ot{m}")
                      for m in range(6)]

                def proj(qn):
                    for m in range(6):
                        pb = psB.tile([128, W2], F32, name="pb", tag="pb")
                        for kc in range(6):
                            nc.tensor.matmul(
                                pb[:],
                                lhsT=wvt[:, kc * D + m * 128:
                                         kc * D + (m + 1) * 128],
                                rhs=sxT[kc][:, qn * W2:(qn + 1) * W2],
                                start=(kc == 0), stop=(kc == 5))
                        eng = nc.vector.tensor_copy if m % 2 else nc.scalar.copy
                        eng(out=ot[m][:, qn * W2:(qn + 1) * W2], in_=pb[:])

                # PE order: front-load gathers, interleave projections
                gather(0)
                gather(1)
                gather(2)
                gather(3)
                proj(0)
                gather(4)
                gather(5)
                proj(1)
                gather(6)
                gather(7)
                proj(2)
                proj(3)
                for m in range(6):
                    nc.sync.dma_start(out=o_d[:, m * Q:(m + 1) * Q],
                                      in_=ot[m][:])

            if iters == 1:
                body()
            else:
                with tc.For_i(0, iters, 1):
                    body()


def _build(iters=1, cfg=None):
    key = (iters, cfg)
    if key not in _NC:
        nc = bacc.Bacc("TRN2", target_bir_lowering=False, debug=False,
                       num_devices=N_CORES)
        _emit(nc, iters, cfg)
        nc.compile()
        _NC[key] = nc
    return _NC[key]


def _sample_params(img_ids, avgs, std_devs, noise):
    """Per-(b,p) bilinear tap indices and weights, replicating the
    reference's fp32 math (int32 truncation + positive mod)."""
    ids = np.asarray(img_ids).astype(np.int64)
    a = np.asarray(avgs, np.float32)[ids]        # (B,2,P)
    s = np.asarray(std_devs, np.float32)[ids]
    nz = np.asarray(noise, np.float32)
    kx = (nz[:, 0] - a[:, 0]) / s[:, 0]          # (B,P) f32
    ky = (nz[:, 1] - a[:, 1]) / s[:, 1]
    x1, x2 = np.ceil(kx), np.floor(kx)
    y1, y2 = np.ceil(ky), np.floor(ky)
    wx1, wx2 = 1.0 - np.abs(x1 - kx), 1.0 - np.abs(x2 - kx)
    wy1, wy2 = 1.0 - np.abs(y1 - ky), 1.0 - np.abs(y2 - ky)
    taps = []
    for px, wx in ((x1, wx1), (x2, wx2)):
        for py, wy in ((y1, wy1), (y2, wy2)):
            idx = (np.float32(GRID) * py + px).astype(np.int32) % S
            taps.append((idx, (wx * wy).astype(np.float32)))
    return taps


def _pack_inputs(x, img_ids, Wv, avgs, std_devs, noise):
    global _CFG, _ORDER, _PERM
    x = np.asarray(x, np.float32)
    wv = np.asarray(Wv, np.float32)
    wvp = np.ascontiguousarray(
        wv.reshape(6, 128, D).transpose(1, 0, 2).reshape(128, 6 * D)
    ).astype(np.float16)

    taps = _sample_params(img_ids, avgs, std_devs, noise)
    idxs = np.stack([t[0] for t in taps])       # (4,B,P)
    lo_t, hi_t = idxs.min(axis=0), idxs.max(axis=0)   # (B,P)
    is_a = hi_t < SPLIT                          # all taps in chunk A
    clean = bool(np.all(is_a | (lo_t >= SPLIT)))

    # dense per-image gather matrices GT[s, p]
    G = np.zeros((B, S, P), np.float32)
    bidx = np.arange(B)[:, None]
    pidx = np.arange(P)[None, :]
    for idx, w in taps:
        np.add.at(G, (bidx, idx, pidx), w)

    if clean:
        # permute columns per image: chunk-A columns first; sort images by
        # Na so the 8 SPMD cores share per-slot matmul shapes
        perm = np.argsort(~is_a, axis=1, kind="stable")     # (B,P)
        na = is_a.sum(axis=1)                               # (B,)
        order = np.argsort(na, kind="stable")               # ranks
        cfg = tuple(
            (int(na[order[8 * b:8 * b + 8]].min()),
             int(na[order[8 * b:8 * b + 8]].max()))
            for b in range(BPC))
        G = np.take_along_axis(G, perm[:, None, :], axis=2)
        s0, s1 = SPLIT, S - SPLIT                           # 98, 99 rows
    else:
        perm = np.broadcast_to(np.arange(P), (B, P))
        order = np.arange(B)
        cfg = None
        s0, s1 = 128, 69
    _CFG, _ORDER, _PERM = cfg, order, perm
    G16 = G.astype(np.float16)

    in_maps = []
    for c in range(N_CORES):
        imgs = [int(order[8 * b + c]) if clean else c * BPC + b
                for b in range(BPC)]
        xt = np.zeros((2 * BPC, 128, D), np.float16)
        gp = np.zeros((128, BPC * W2), np.float16)
        for b, im in enumerate(imgs):
            xt[2 * b, :s0] = x[im, 0:s0]
            xt[2 * b + 1, :s1] = x[im, s0:S]
            gp[:s0, b * W2:b * W2 + P] = G16[im, 0:s0]
            gp[:s1, b * W2 + P:(b + 1) * W2] = G16[im, s0:S]
        xp = np.ascontiguousarray(
            xt.transpose(1, 0, 2).reshape(128, 2 * BPC * D))
        in_maps.append({"x0": xp, "wv0": wvp, "gt0": gp})
    return in_maps


def _unpack_out(o_np, c, out):
    # o_np: (128, 6*Q) fp16 = out^T chunks; scatter into out (B,S,D)
    svT = o_np.reshape(128, 6, Q).transpose(1, 0, 2).reshape(D, Q)
    sv = svT.T.astype(np.float32).reshape(BPC, P, D)
    for b in range(BPC):
        im = int(_ORDER[8 * b + c]) if _CFG else c * BPC + b
        out[im, 1 + _PERM[im], :] = sv[b]


def _get_runner(iters=1):
    """Build the sharded PJRT callable once and cache it."""
    key = (iters, _CFG)
    if key in _RUNNER:
        return _RUNNER[key]
    import jax
    from jax.experimental.shard_map import shard_map
    from jax.sharding import Mesh, PartitionSpec
    from concourse import bass2jax, mybir as _mybir

    nc = _build(iters, _CFG)
    bass2jax.install_neuronx_cc_hook()
    in_names, out_names, out_avals, zero_outs = [], [], [], []
    part_name = (nc.partition_id_tensor.name
                 if nc.partition_id_tensor else None)
    for alloc in nc.m.functions[0].allocations:
        if not isinstance(alloc, _mybir.MemoryLocationSet):
            continue
        name = alloc.memorylocations[0].name
        if alloc.kind == "ExternalInput":
            if name != part_name:
                in_names.append(name)
        elif alloc.kind == "ExternalOutput":
            shape = tuple(alloc.tensor_shape)
            dtype = _mybir.dt.np(alloc.dtype)
            out_names.append(name)
            out_avals.append(jax.core.ShapedArray(shape, dtype))
            zero_outs.append(np.zeros(shape, dtype))
    n_params = len(in_names)
    all_names = in_names + out_names
    if part_name is not None:
        all_names = all_names + [part_name]
    donate = tuple(range(n_params, n_params + len(out_names)))

    def _body(*args):
        operands = list(args)
        if part_name is not None:
            operands.append(bass2jax.partition_id_tensor())
        outs = bass2jax._bass_exec_p.bind(
            *operands,
            out_avals=tuple(out_avals),
            in_names=tuple(all_names),
            out_names=tuple(out_names),
            lowering_input_output_aliases=(),
            sim_require_finite=True,
            sim_require_nnan=True,
            nc=nc,
        )
        return tuple(outs)

    devices = jax.devices()[:N_CORES]
    mesh = Mesh(np.asarray(devices), ("core",))
    specs = (PartitionSpec("core"),) * (n_params + len(out_names))
    fn = jax.jit(
        shard_map(_body, mesh=mesh, in_specs=specs,
                  out_specs=(PartitionSpec("core"),) * len(out_names),
                  check_rep=False),
        donate_argnums=donate, keep_unused=True)

    def run(in_maps):
        concat_in = [
            np.concatenate([np.asarray(m[nm]) for m in in_maps], axis=0)
            for nm in in_names
        ]
        concat_zero = [
            np.zeros((N_CORES * z.shape[0], *z.shape[1:]), z.dtype)
            for z in zero_outs
        ]
        arrs = fn(*concat_in, *concat_zero)
        return [
            {nm: np.asarray(arrs[i]).reshape(N_CORES, *out_avals[i].shape)[c]
             for i, nm in enumerate(out_names)}
            for c in range(N_CORES)
        ]

    _RUNNER[key] = run
    return run


class _Res:
    def __init__(self, results):
        self.results = results
        self.exec_time_ns = None


def run_cores(in_maps, trace=False, iters=1):
    return _Res(_get_runner(iters)(in_maps))


def kernel(x, img_ids, mask=None, Wq=None, bq=None, Wk=None, bk=None,
           Wv=None, bv=None, avgs=None, std_devs=None, noise=None,
           _trace=False, _results=None):
    in_maps = _pack_inputs(x, img_ids, Wv, avgs, std_devs, noise)
    res = run_cores(in_maps, trace=_trace)
    if _results is not None:
        _results.append(res)
    out = np.ones((B, S, D), np.float32)
    for c in range(N_CORES):
        _unpack_out(res.results[c]["o0"], c, out)
    bv_np = np.asarray(bv, np.float32) if bv is not None else None
    if bv_np is not None and np.any(bv_np):
        # sample() is affine: add (sum_i w_i) * bv for the sampled rows.
        wsum = np.zeros((B, P), np.float32)
        for _, w in _sample_params(img_ids, avgs, std_devs, noise):
            wsum += w
        out[:, 1:, :] += wsum[:, :, None] * bv_np[None, None, :]
    return out
